# revision 4
# baseline (speedup 1.0000x reference)
"""Contrastive loss (InfoNCE-style logsumexp of cosine-similarity matrix) on
8 Trainium2 NeuronCores — fp8 DoubleRow edition.

loss = -mean_i logsumexp_j( cos(z1_i, z2_j) / 0.05 ),  z1,z2: [8192, 512] f32

Strategy: shard z1 row-wise (1024 rows/core), replicate z2. All matmul
operands in float8e4 (e4m3): numerics validated at ~5e-5 rel err (tolerance
2e-2) because per-element quantization noise averages out over the K=512 dot
and the 8192-term logsumexp.

Per core:
  1. z2 streams in via gpsimd cast-DMAs (f32 DRAM -> bf16 SBUF, RNE);
     row sum-of-squares as bf16 square+reduce on DVE (2x 16-bit mode);
     1/||z2|| via quake-rsqrt (bitcast + 3 Newton steps on DVE — keeps
     ACT's activation table untouched); normalize-and-cast to fp8 scaled
     by 16 (tensor_scalar, split DVE/Pool) so values sit in e4m3's normal
     range,
  2. fp8 PE transposes (1 cyc/row): hw writes fp8 transpose outputs at
     element step 2, so transposed operands use a stride-2 interleaved
     layout end-to-end; PSUM -> SBUF drains are bitcast-f32 copies
     (4 bytes/cycle/lane),
  3. sim block-rows: DoubleRow fp8 matmuls (two k-tiles per instruction,
     2x bf16 throughput) into [128, 2048] 4-bank PSUM tiles, two tiles
     ping-pong across the 8 banks; transposes for the next j-supergroup
     slot into the same pool ring between main tiles,
  4. one ACT Exp per [128, 2048] tile, in place, scale = 1.25/||z1_i||
     (= 20/(16*||z1_i||), folding the fp8 pre-scale), fused row-sum via
     accum_out. Exp/Ln/Square/Copy share one act table -> one table load;
     z1's prep (sum-of-squares + fp8 casts) runs on ACT in the startup
     window before the first Exp.
  5. reduce + Ln -> per-row lse [128, 8] -> DRAM. Host gathers, -mean.
"""
import sys

sys.path.insert(0, "/opt/trn_rl_repo")
import numpy as np
import concourse.bacc as bacc
import concourse.mybir as mybir
from concourse import tile, masks
from concourse.bass_utils import run_bass_kernel_spmd

F32 = mybir.dt.float32
BF16 = mybir.dt.bfloat16
FP8 = mybir.dt.float8e4
U32 = mybir.dt.uint32
AF = mybir.ActivationFunctionType
ALU = mybir.AluOpType
DR = mybir.MatmulPerfMode.DoubleRow

N, D, C = 8192, 512, 8
NS = N // C            # 1024 z1 rows per core
IB = NS // 128         # 8 i-blocks per core
NJH = 4                # j-supergroups of 2048 columns
QK = 0x5F3759DF        # quake rsqrt magic

# CoreSim-only: zero-fill transpose PSUM tiles so the bitcast copy over the
# stride-2 gaps doesn't trip the interpreter's uninitialized-read check.
# Hardware never reads those bytes meaningfully; keep False for real runs.
SIM_SAFE = False


def _build():
    nc = bacc.Bacc("TRN2", target_bir_lowering=False, debug=False, num_devices=C)
    z1_d = nc.dram_tensor("z1s", [NS, D], F32, kind="ExternalInput").ap()
    z2_d = nc.dram_tensor("z2", [N, D], F32, kind="ExternalInput").ap()
    lse_d = nc.dram_tensor("lse", [128, IB], F32, kind="ExternalOutput").ap()

    with tile.TileContext(nc) as tc:
        with (
            tc.tile_pool(name="const", bufs=1) as cpool,
            tc.tile_pool(name="stage", bufs=3) as stg,
            tc.tile_pool(name="h8", bufs=12) as h8p,
            tc.tile_pool(name="sq", bufs=3) as sqp,
            tc.tile_pool(name="qk", bufs=3) as qkp,
            tc.tile_pool(name="pbig", bufs=2, space="PSUM") as pbig,
        ):
            ident8 = cpool.tile([128, 128], FP8)
            masks.make_identity(nc, ident8[:])
            Kq = cpool.tile([128, 16], U32, name="Kq")
            nc.gpsimd.memset(Kq[:], QK)

            # stride-2 interleaved transposed operands (see module docstring)
            z2T8 = cpool.tile([128, 4, 2 * N], FP8, name="z2T8")
            z1T8 = cpool.tile([128, 4, 2 * NS], FP8, name="z1T8")
            z2T8v = z2T8[:].rearrange("p k (j t) -> p k j t", t=2)
            z1T8v = z1T8[:].rearrange("p k (j t) -> p k j t", t=2)

            n2sq = cpool.tile([128, 64], F32, name="n2sq")
            rn2 = cpool.tile([128, 64], F32, name="rn2")
            n1sq = cpool.tile([128, IB], F32, name="n1sq")
            rn1 = cpool.tile([128, IB], F32, name="rn1")
            esums = cpool.tile([128, NJH * IB], F32, name="esums")
            stot = cpool.tile([128, IB], F32, name="stot")
            lse_s = cpool.tile([128, IB], F32, name="lse_s")
            st1 = cpool.tile([128, IB, D], BF16, name="st1")  # z1 bf16 stage

            def rsqrt_cols(dst, src, w, scale):
                # dst = scale / sqrt(src) entirely on DVE (no ACT table load):
                # quake initial guess + 3 Newton steps (~1e-7 rel err)
                tu = qkp.tile([128, 16], U32, tag="qu", name="tu")
                y = qkp.tile([128, 16], F32, tag="qy", name="qy")
                a = qkp.tile([128, 16], F32, tag="qa", name="qa")
                nc.vector.tensor_scalar(
                    tu[:, :w], src.bitcast(U32), 1, None,
                    op0=ALU.logical_shift_right)
                nc.vector.tensor_sub(y[:, :w].bitcast(U32), Kq[:, :w], tu[:, :w])
                for _ in range(3):
                    nc.vector.tensor_mul(a[:, :w], y[:, :w], y[:, :w])
                    nc.vector.tensor_mul(a[:, :w], a[:, :w], src)
                    nc.vector.tensor_scalar(
                        a[:, :w], a[:, :w], -0.5, 1.5, op0=ALU.mult, op1=ALU.add)
                    nc.vector.tensor_mul(y[:, :w], y[:, :w], a[:, :w])
                nc.vector.tensor_scalar(dst, y[:, :w], float(scale), None,
                                        op0=ALU.mult)

            # ---------------- z2 streaming machinery
            z2r = z2_d.rearrange("(s n p) d -> s p n d", n=4, p=128)  # 16 stages
            z2st, z2h8 = {}, {}

            def z2_stage(s):
                # cast-DMA: f32 DRAM -> bf16 SBUF (gpsimd software DGE)
                st = stg.tile([128, 4, D], BF16, tag="st", name=f"st2_{s}")
                nc.gpsimd.dma_start(out=st[:], in_=z2r[s])
                z2st[s] = st
                for n in range(4):
                    b = 4 * s + n
                    sq = sqp.tile([128, D], BF16, tag="sq", name="sq_scr")
                    nc.vector.tensor_mul(sq[:], st[:, n, :], st[:, n, :])
                    nc.vector.tensor_reduce(n2sq[:, b:b + 1], sq[:],
                                            axis=mybir.AxisListType.X,
                                            op=ALU.add)

            def z2_norm8(h):
                # blocks 8h..8h+7: rsqrt batch then normalize+cast to fp8*16
                rsqrt_cols(rn2[:, 8 * h:8 * h + 8], n2sq[:, 8 * h:8 * h + 8],
                           8, 16.0)
                for b in range(8 * h, 8 * h + 8):
                    s, n = divmod(b, 4)
                    st = z2st[s]
                    zh = h8p.tile([128, D], FP8, tag="h8", name=f"zh{b}")
                    eng = nc.vector if (b % 4 == 0) else nc.gpsimd
                    eng.tensor_scalar(zh[:], st[:, n, :], rn2[:, b:b + 1], None,
                                      op0=ALU.mult)
                    z2h8[b] = zh

            def z2_T(h):
                # 32 fp8 transposes (stride-2 out) -> 4-bank PSUM tile ->
                # z2T8 cols, moved as a bitcast f32 copy (4 bytes/elem)
                T = pbig.tile([128, 4, 2048], FP8, tag="big", name=f"T{h}")
                Tv = T[:].rearrange("p k (j t) -> p k j t", t=2)
                if SIM_SAFE:
                    nc.gpsimd.memset(T[:].bitcast(F32), 0.0)
                for k in range(4):
                    for bi in range(8):
                        zh = z2h8[8 * h + bi]
                        nc.tensor.transpose(
                            Tv[:, k, bi * 128:(bi + 1) * 128, 0],
                            zh[:, k * 128:(k + 1) * 128], ident8[:])
                nc.vector.tensor_copy(
                    z2T8[:, :, h * 2048:(h + 1) * 2048].bitcast(F32),
                    T[:].bitcast(F32))
                for bi in range(8):
                    del z2h8[8 * h + bi]

            # ---------------- z1 startup (sumsq + casts on ACT: its table-
            # free Square/Copy run in the pre-Exp idle window)
            z1r = z1_d.rearrange("(b p) d -> p b d", p=128)
            z1h8 = {}

            def z1_prep():
                nc.gpsimd.dma_start(out=st1[:], in_=z1r)
                for b in range(IB):
                    sq = sqp.tile([128, D], BF16, tag="sq", name="sq1_scr")
                    nc.scalar.activation(sq[:], st1[:, b, :], AF.Square,
                                         accum_out=n1sq[:, b:b + 1])
                    zh = h8p.tile([128, D], FP8, tag="h8", name=f"z1h{b}")
                    nc.scalar.copy(zh[:], st1[:, b, :])
                    z1h8[b] = zh
                # 1.25 = 20 (1/T) / 16 (fp8 pre-scale on z2hat)
                rsqrt_cols(rn1[:], n1sq[:], IB, 1.25)
                T = pbig.tile([128, 4, 2048], FP8, tag="big", name="T1")
                Tv = T[:].rearrange("p k (j t) -> p k j t", t=2)
                if SIM_SAFE:
                    nc.gpsimd.memset(T[:].bitcast(F32), 0.0)
                for k in range(4):
                    for bi in range(IB):
                        nc.tensor.transpose(
                            Tv[:, k, bi * 128:(bi + 1) * 128, 0],
                            z1h8[bi][:, k * 128:(k + 1) * 128], ident8[:])
                nc.vector.tensor_copy(z1T8[:].bitcast(F32), T[:].bitcast(F32))

            # ---------------- main tiles
            def main_tile(ib, jh):
                ps = pbig.tile([128, 2048], F32, tag="big", name=f"mm{ib}_{jh}")
                psv = ps[:].rearrange("p (jq x) -> p jq x", jq=4)
                for kp in range(2):
                    lhsT = z1T8v[:, 2 * kp:2 * kp + 2,
                                 ib * 128:(ib + 1) * 128, 0]
                    for jq in range(4):
                        rhs = z2T8v[:, 2 * kp:2 * kp + 2,
                                    jh * 2048 + jq * 512:jh * 2048 + (jq + 1) * 512,
                                    0]
                        nc.tensor.matmul(
                            psv[:, jq, :], lhsT=lhsT, rhs=rhs,
                            start=(kp == 0), stop=(kp == 1),
                            perf_mode=DR, skip_group_check=True)
                nc.scalar.activation(
                    ps[:], ps[:], AF.Exp, scale=rn1[:, ib:ib + 1],
                    accum_out=esums[:, jh * IB + ib:jh * IB + ib + 1])

            # ---------------- emission
            z2_stage(0)
            z2_stage(1)
            z1_prep()
            z2_norm8(0)
            z2_T(0)
            z2_stage(2)
            z2_stage(3)
            z2_norm8(1)
            z2_T(1)
            for jh in range(NJH):
                for ib in range(IB):
                    main_tile(ib, jh)
                    if jh < NJH - 1:
                        s0 = 4 * (jh + 1)
                        if ib == 0:
                            z2_stage(s0)
                        elif ib == 1:
                            z2_stage(s0 + 1)
                        elif ib == 2:
                            z2_norm8(2 * (jh + 1))
                        elif ib == 3:
                            z2_T(2 * (jh + 1))
                        elif ib == 4:
                            z2_stage(s0 + 2)
                        elif ib == 5:
                            z2_stage(s0 + 3)
                        elif ib == 6:
                            z2_norm8(2 * (jh + 1) + 1)
                        elif ib == 7:
                            z2_T(2 * (jh + 1) + 1)

            # ---------------- logsumexp tail
            nc.vector.reduce_sum(
                stot[:], esums[:].rearrange("p (a b) -> p b a", a=NJH),
                axis=mybir.AxisListType.X)
            nc.scalar.activation(lse_s[:], stot[:], AF.Ln)
            nc.sync.dma_start(out=lse_d[:], in_=lse_s[:])

    nc.compile()
    return nc


_nc = None


def _get_nc():
    global _nc
    if _nc is None:
        _nc = _build()
    return _nc


def kernel(z1: np.ndarray, z2: np.ndarray, _trace: bool = False, **_):
    nc = _get_nc()
    z1 = np.ascontiguousarray(z1, dtype=np.float32)
    z2 = np.ascontiguousarray(z2, dtype=np.float32)
    in_maps = [
        {"z1s": z1[c * NS:(c + 1) * NS], "z2": z2} for c in range(C)
    ]
    res = run_bass_kernel_spmd(nc, in_maps, list(range(C)), trace=_trace)
    total = 0.0
    for c in range(C):
        total += res.results[c]["lse"].astype(np.float64).sum()
    out = np.float32(-(total / N))
    if _trace:
        return out, res
    return out


# revision 5
# speedup vs baseline: 1.0848x; 1.0848x over previous
"""Contrastive loss (InfoNCE-style logsumexp of cosine-similarity matrix) on
8 Trainium2 NeuronCores — fp8 DoubleRow edition.

loss = -mean_i logsumexp_j( cos(z1_i, z2_j) / 0.05 ),  z1,z2: [8192, 512] f32

Strategy: shard z1 row-wise (1024 rows/core), replicate z2. All matmul
operands in float8e4 (e4m3): numerics validated at ~5e-5 rel err (tolerance
2e-2) because per-element quantization noise averages out over the K=512 dot
and the 8192-term logsumexp.

Per core:
  1. z2 streams in via gpsimd cast-DMAs (f32 DRAM -> bf16 SBUF, RNE);
     per-stage bf16 squares + per-block tensor_scalar-with-accum row sums
     on DVE (2x 16-bit mode); 1/||z2|| via quake-rsqrt (bitcast + Newton
     on DVE — keeps ACT's activation table untouched); normalize in bf16
     scaled by 16 (so values sit in e4m3's normal range), then ONE batched
     SBUF->SBUF cast-DMA per 8 blocks converts bf16 -> fp8 off-engine
     (vector-engine fp8 converts are microcode-slow: measured 4-8us/block),
  2. fp8 PE transposes (1 cyc/row): hw writes fp8 transpose outputs at
     element step 2, so transposed operands use a stride-2 interleaved
     layout end-to-end; PSUM -> SBUF drains are bitcast-f32 copies
     (4 bytes/cycle/lane),
  3. sim block-rows: DoubleRow fp8 matmuls (two k-tiles per instruction,
     2x bf16 throughput) into [128, 2048] 4-bank PSUM tiles, two tiles
     ping-pong across the 8 banks; transposes for the next j-supergroup
     slot into the same pool ring between main tiles,
  4. one ACT Exp per [128, 2048] tile, in place, scale = 1.25/||z1_i||
     (= 20/(16*||z1_i||), folding the fp8 pre-scale), fused row-sum via
     accum_out. Exp/Ln/Square/Copy share one act table -> one table load;
     z1's prep (sum-of-squares + fp8 casts) runs on ACT in the startup
     window before the first Exp.
  5. reduce + Ln -> per-row lse [128, 8] -> DRAM. Host gathers, -mean.
"""
import sys

sys.path.insert(0, "/opt/trn_rl_repo")
import numpy as np
import concourse.bacc as bacc
import concourse.mybir as mybir
from concourse import tile, masks
from concourse.bass_utils import run_bass_kernel_spmd

F32 = mybir.dt.float32
BF16 = mybir.dt.bfloat16
FP8 = mybir.dt.float8e4
U32 = mybir.dt.uint32
AF = mybir.ActivationFunctionType
ALU = mybir.AluOpType
DR = mybir.MatmulPerfMode.DoubleRow

N, D, C = 8192, 512, 8
NS = N // C            # 1024 z1 rows per core
IB = NS // 128         # 8 i-blocks per core
NJH = 4                # j-supergroups of 2048 columns
QK = 0x5F3759DF        # quake rsqrt magic

# CoreSim-only: zero-fill transpose PSUM tiles so the bitcast copy over the
# stride-2 gaps doesn't trip the interpreter's uninitialized-read check.
# Hardware never reads those bytes meaningfully; keep False for real runs.
SIM_SAFE = False


def _build():
    nc = bacc.Bacc("TRN2", target_bir_lowering=False, debug=False, num_devices=C)
    z1_d = nc.dram_tensor("z1s", [NS, D], F32, kind="ExternalInput").ap()
    z2_d = nc.dram_tensor("z2", [N, D], F32, kind="ExternalInput").ap()
    lse_d = nc.dram_tensor("lse", [128, IB], F32, kind="ExternalOutput").ap()

    with tile.TileContext(nc) as tc:
        with (
            tc.tile_pool(name="const", bufs=1) as cpool,
            tc.tile_pool(name="stage", bufs=3) as stg,
            tc.tile_pool(name="hb", bufs=2) as hbp,
            tc.tile_pool(name="h8f", bufs=2) as h8fp,
            tc.tile_pool(name="sq", bufs=2) as sqp,
            tc.tile_pool(name="qk", bufs=3) as qkp,
            tc.tile_pool(name="pbig", bufs=2, space="PSUM") as pbig,
        ):
            ident8 = cpool.tile([128, 128], FP8)
            masks.make_identity(nc, ident8[:])
            Kq = cpool.tile([128, 16], U32, name="Kq")
            nc.gpsimd.memset(Kq[:], QK)

            # stride-2 interleaved transposed operands (see module docstring)
            z2T8 = cpool.tile([128, 4, 2 * N], FP8, name="z2T8")
            z1T8 = cpool.tile([128, 4, 2 * NS], FP8, name="z1T8")
            z2T8v = z2T8[:].rearrange("p k (j t) -> p k j t", t=2)
            z1T8v = z1T8[:].rearrange("p k (j t) -> p k j t", t=2)

            n2sq = cpool.tile([128, 64], F32, name="n2sq")
            rn2 = cpool.tile([128, 64], F32, name="rn2")
            n1sq = cpool.tile([128, IB], F32, name="n1sq")
            rn1 = cpool.tile([128, IB], F32, name="rn1")
            esums = cpool.tile([128, NJH * IB], F32, name="esums")
            stot = cpool.tile([128, IB], F32, name="stot")
            lse_s = cpool.tile([128, IB], F32, name="lse_s")
            st1 = cpool.tile([128, IB, D], BF16, name="st1")  # z1 bf16 stage

            def rsqrt_cols(dst, src, w, scale):
                # dst = scale / sqrt(src) entirely on DVE (no ACT table load):
                # quake initial guess + 2 Newton steps (~5e-6 rel err)
                tu = qkp.tile([128, 16], U32, tag="qu", name="tu")
                y = qkp.tile([128, 16], F32, tag="qy", name="qy")
                a = qkp.tile([128, 16], F32, tag="qa", name="qa")
                nc.vector.tensor_scalar(
                    tu[:, :w], src.bitcast(U32), 1, None,
                    op0=ALU.logical_shift_right)
                nc.vector.tensor_sub(y[:, :w].bitcast(U32), Kq[:, :w], tu[:, :w])
                for _ in range(2):
                    nc.vector.tensor_mul(a[:, :w], y[:, :w], y[:, :w])
                    nc.vector.tensor_mul(a[:, :w], a[:, :w], src)
                    nc.vector.tensor_scalar(
                        a[:, :w], a[:, :w], -0.5, 1.5, op0=ALU.mult, op1=ALU.add)
                    nc.vector.tensor_mul(y[:, :w], y[:, :w], a[:, :w])
                nc.vector.tensor_scalar(dst, y[:, :w], float(scale), None,
                                        op0=ALU.mult)

            # ---------------- z2 streaming machinery
            z2r = z2_d.rearrange("(s n p) d -> s p n d", n=4, p=128)  # 16 stages
            z2st, z2h8 = {}, {}

            def z2_stage(s):
                # cast-DMA: f32 DRAM -> bf16 SBUF (gpsimd software DGE),
                # then one wide bf16 square (DVE 2x mode) + per-block
                # tensor_scalar row-sum accumulation
                st = stg.tile([128, 4, D], BF16, tag="st", name=f"st2_{s}")
                nc.gpsimd.dma_start(out=st[:], in_=z2r[s])
                z2st[s] = st
                sq = sqp.tile([128, 4, D], BF16, tag="sq", name="sq_scr")
                nc.vector.tensor_mul(sq[:], st[:], st[:])
                for n in range(4):
                    b = 4 * s + n
                    nc.vector.tensor_scalar(
                        sq[:, n, :], sq[:, n, :], 1.0, 0.0, op0=ALU.mult,
                        op1=ALU.add, accum_out=n2sq[:, b:b + 1])

            def z2_norm8(h):
                # blocks 8h..8h+7: rsqrt batch, normalize in bf16 (DVE/Pool
                # split), then one batched cast-DMA bf16 -> fp8
                rsqrt_cols(rn2[:, 8 * h:8 * h + 8], n2sq[:, 8 * h:8 * h + 8],
                           8, 16.0)
                hb = hbp.tile([128, 8, D], BF16, tag="hb", name=f"hb{h}")
                for bi in range(8):
                    b = 8 * h + bi
                    s, n = divmod(b, 4)
                    st = z2st[s]
                    eng = nc.vector if bi in (0, 3, 6) else nc.gpsimd
                    eng.tensor_scalar(hb[:, bi, :], st[:, n, :],
                                      rn2[:, b:b + 1], None, op0=ALU.mult)
                h8 = h8fp.tile([128, 8, D], FP8, tag="h8f", name=f"h8f{h}")
                nc.gpsimd.dma_start(out=h8[:], in_=hb[:])
                z2h8[h] = h8

            def z2_T(h):
                # 32 fp8 transposes (stride-2 out) -> 4-bank PSUM tile ->
                # z2T8 cols, moved as a bitcast f32 copy (4 bytes/elem)
                h8 = z2h8.pop(h)
                T = pbig.tile([128, 4, 2048], FP8, tag="big", name=f"T{h}")
                Tv = T[:].rearrange("p k (j t) -> p k j t", t=2)
                if SIM_SAFE:
                    nc.gpsimd.memset(T[:].bitcast(F32), 0.0)
                for k in range(4):
                    for bi in range(8):
                        nc.tensor.transpose(
                            Tv[:, k, bi * 128:(bi + 1) * 128, 0],
                            h8[:, bi, k * 128:(k + 1) * 128], ident8[:])
                nc.vector.tensor_copy(
                    z2T8[:, :, h * 2048:(h + 1) * 2048].bitcast(F32),
                    T[:].bitcast(F32))

            # ---------------- z1 startup (sumsq + fp8 casts on ACT: its
            # table-free Square/Copy run in the pre-Exp idle window)
            z1r = z1_d.rearrange("(b p) d -> p b d", p=128)
            z1h8f = cpool.tile([128, IB, D], FP8, name="z1h8f")

            def z1_prep():
                nc.gpsimd.dma_start(out=st1[:], in_=z1r)
                for b in range(IB):
                    sq = sqp.tile([128, 4, D], BF16, tag="sq", name="sq1_scr")
                    nc.scalar.activation(sq[:, 0, :], st1[:, b, :], AF.Square,
                                         accum_out=n1sq[:, b:b + 1])
                    nc.scalar.copy(z1h8f[:, b, :], st1[:, b, :])
                # 1.25 = 20 (1/T) / 16 (fp8 pre-scale on z2hat)
                rsqrt_cols(rn1[:], n1sq[:], IB, 1.25)
                T = pbig.tile([128, 4, 2048], FP8, tag="big", name="T1")
                Tv = T[:].rearrange("p k (j t) -> p k j t", t=2)
                if SIM_SAFE:
                    nc.gpsimd.memset(T[:].bitcast(F32), 0.0)
                for k in range(4):
                    for bi in range(IB):
                        nc.tensor.transpose(
                            Tv[:, k, bi * 128:(bi + 1) * 128, 0],
                            z1h8f[:, bi, k * 128:(k + 1) * 128], ident8[:])
                nc.vector.tensor_copy(z1T8[:].bitcast(F32), T[:].bitcast(F32))

            # ---------------- main tiles
            def main_tile(ib, jh):
                ps = pbig.tile([128, 2048], F32, tag="big", name=f"mm{ib}_{jh}")
                psv = ps[:].rearrange("p (jq x) -> p jq x", jq=4)
                for kp in range(2):
                    lhsT = z1T8v[:, 2 * kp:2 * kp + 2,
                                 ib * 128:(ib + 1) * 128, 0]
                    for jq in range(4):
                        rhs = z2T8v[:, 2 * kp:2 * kp + 2,
                                    jh * 2048 + jq * 512:jh * 2048 + (jq + 1) * 512,
                                    0]
                        nc.tensor.matmul(
                            psv[:, jq, :], lhsT=lhsT, rhs=rhs,
                            start=(kp == 0), stop=(kp == 1),
                            perf_mode=DR, skip_group_check=True)
                nc.scalar.activation(
                    ps[:], ps[:], AF.Exp, scale=rn1[:, ib:ib + 1],
                    accum_out=esums[:, jh * IB + ib:jh * IB + ib + 1])

            # ---------------- emission
            z2_stage(0)
            z2_stage(1)
            z1_prep()
            z2_norm8(0)
            z2_T(0)
            z2_stage(2)
            z2_stage(3)
            z2_norm8(1)
            z2_T(1)
            for jh in range(NJH):
                for ib in range(IB):
                    main_tile(ib, jh)
                    if jh < NJH - 1:
                        s0 = 4 * (jh + 1)
                        if ib == 0:
                            z2_stage(s0)
                        elif ib == 1:
                            z2_stage(s0 + 1)
                        elif ib == 2:
                            z2_norm8(2 * (jh + 1))
                        elif ib == 3:
                            z2_T(2 * (jh + 1))
                        elif ib == 4:
                            z2_stage(s0 + 2)
                        elif ib == 5:
                            z2_stage(s0 + 3)
                        elif ib == 6:
                            z2_norm8(2 * (jh + 1) + 1)
                        elif ib == 7:
                            z2_T(2 * (jh + 1) + 1)

            # ---------------- logsumexp tail
            nc.vector.reduce_sum(
                stot[:], esums[:].rearrange("p (a b) -> p b a", a=NJH),
                axis=mybir.AxisListType.X)
            nc.scalar.activation(lse_s[:], stot[:], AF.Ln)
            nc.sync.dma_start(out=lse_d[:], in_=lse_s[:])

    nc.compile()
    return nc


_nc = None


def _get_nc():
    global _nc
    if _nc is None:
        _nc = _build()
    return _nc


def kernel(z1: np.ndarray, z2: np.ndarray, _trace: bool = False, **_):
    nc = _get_nc()
    z1 = np.ascontiguousarray(z1, dtype=np.float32)
    z2 = np.ascontiguousarray(z2, dtype=np.float32)
    in_maps = [
        {"z1s": z1[c * NS:(c + 1) * NS], "z2": z2} for c in range(C)
    ]
    res = run_bass_kernel_spmd(nc, in_maps, list(range(C)), trace=_trace)
    total = 0.0
    for c in range(C):
        total += res.results[c]["lse"].astype(np.float64).sum()
    out = np.float32(-(total / N))
    if _trace:
        return out, res
    return out


# revision 8
# speedup vs baseline: 2.2446x; 2.0692x over previous
"""Contrastive loss (InfoNCE-style logsumexp of cosine-similarity matrix) on
8 Trainium2 NeuronCores — fp8 DoubleRow edition.

loss = -mean_i logsumexp_j( cos(z1_i, z2_j) / 0.05 ),  z1,z2: [8192, 512] f32

Strategy: shard z1 row-wise (1024 rows/core), replicate z2. All matmul
operands in float8e4 (e4m3): numerics validated at ~5e-5 rel err (tolerance
2e-2) because per-element quantization noise averages out over the K=512 dot
and the 8192-term logsumexp.

Per core:
  1. z2 streams in via gpsimd cast-DMAs (f32 DRAM -> bf16 SBUF, RNE);
     per-stage bf16 squares + per-block tensor_scalar-with-accum row sums
     on DVE (2x 16-bit mode); 1/||z2|| via quake-rsqrt (bitcast + Newton
     on DVE — keeps ACT's activation table untouched); normalize in bf16
     scaled by 16 (so values sit in e4m3's normal range), then ONE batched
     SBUF->SBUF cast-DMA per 8 blocks converts bf16 -> fp8 off-engine
     (vector-engine fp8 converts are microcode-slow: measured 4-8us/block),
  2. fp8 PE transposes (1 cyc/row): hw writes fp8 transpose outputs at
     element step 2, so transposed operands use a stride-2 interleaved
     layout end-to-end; PSUM -> SBUF drains are bitcast-f32 copies
     (4 bytes/cycle/lane),
  3. sim block-rows: DoubleRow fp8 matmuls (two k-tiles per instruction,
     2x bf16 throughput) into [128, 2048] 4-bank PSUM tiles, two tiles
     ping-pong across the 8 banks; transposes for the next j-supergroup
     slot into the same pool ring between main tiles,
  4. one ACT Exp per [128, 2048] tile, in place, scale = 1.25/||z1_i||
     (= 20/(16*||z1_i||), folding the fp8 pre-scale), fused row-sum via
     accum_out. Exp/Ln/Square/Copy share one act table -> one table load;
     z1's prep (sum-of-squares + fp8 casts) runs on ACT in the startup
     window before the first Exp.
  5. reduce + Ln -> per-row lse [128, 8] -> DRAM. Host gathers, -mean.
"""
import sys

sys.path.insert(0, "/opt/trn_rl_repo")
import numpy as np
import concourse.bacc as bacc
import concourse.mybir as mybir
from concourse import tile, masks
from concourse.bass_utils import run_bass_kernel_spmd

F32 = mybir.dt.float32
BF16 = mybir.dt.bfloat16
FP8 = mybir.dt.float8e4
U32 = mybir.dt.uint32
AF = mybir.ActivationFunctionType
ALU = mybir.AluOpType
DR = mybir.MatmulPerfMode.DoubleRow

N, D, C = 8192, 512, 8
NS = N // C            # 1024 z1 rows per core
IB = NS // 128         # 8 i-blocks per core
NJH = 4                # j-supergroups of 2048 columns
QK = 0x5F3759DF        # quake rsqrt magic

# CoreSim-only: zero-fill transpose PSUM tiles so the bitcast copy over the
# stride-2 gaps doesn't trip the interpreter's uninitialized-read check.
# Hardware never reads those bytes meaningfully; keep False for real runs.
SIM_SAFE = False


def _build():
    nc = bacc.Bacc("TRN2", target_bir_lowering=False, debug=False, num_devices=C)
    z1_d = nc.dram_tensor("z1s", [NS, D], F32, kind="ExternalInput").ap()
    z2_d = nc.dram_tensor("z2", [N, D], F32, kind="ExternalInput").ap()
    lse_d = nc.dram_tensor("lse", [128, IB], F32, kind="ExternalOutput").ap()

    with tile.TileContext(nc) as tc:
        with (
            tc.tile_pool(name="const", bufs=1) as cpool,
            tc.tile_pool(name="stage", bufs=3) as stg,
            tc.tile_pool(name="hb", bufs=2) as hbp,
            tc.tile_pool(name="h8f", bufs=2) as h8fp,
            tc.tile_pool(name="sq", bufs=2) as sqp,
            tc.tile_pool(name="qk", bufs=3) as qkp,
            tc.tile_pool(name="pbig", bufs=2, space="PSUM") as pbig,
        ):
            ident8 = cpool.tile([128, 128], FP8)
            masks.make_identity(nc, ident8[:])
            Kq = cpool.tile([128, 16], U32, name="Kq")
            nc.gpsimd.memset(Kq[:], QK)

            # stride-2 interleaved transposed operands (see module docstring)
            z2T8 = cpool.tile([128, 4, 2 * N], FP8, name="z2T8")
            z1T8 = cpool.tile([128, 4, 2 * NS], FP8, name="z1T8")
            z2T8v = z2T8[:].rearrange("p k (j t) -> p k j t", t=2)
            z1T8v = z1T8[:].rearrange("p k (j t) -> p k j t", t=2)

            n2sq = cpool.tile([128, 64], F32, name="n2sq")
            rn2 = cpool.tile([128, 64], F32, name="rn2")
            n1sq = cpool.tile([128, IB], F32, name="n1sq")
            rn1 = cpool.tile([128, IB], F32, name="rn1")
            esums = cpool.tile([128, NJH * IB], F32, name="esums")
            stot = cpool.tile([128, IB], F32, name="stot")
            lse_s = cpool.tile([128, IB], F32, name="lse_s")
            st1 = cpool.tile([128, IB, D], BF16, name="st1")  # z1 bf16 stage

            def rsqrt_cols(dst, src, w, scale, newton=2):
                # dst = scale / sqrt(src) entirely on DVE (no ACT table load):
                # quake initial guess + Newton steps (1 step ~1.7e-3 rel err,
                # 2 steps ~5e-6; either is far inside the fp8 noise floor)
                tu = qkp.tile([128, 16], U32, tag="qu", name="tu")
                y = qkp.tile([128, 16], F32, tag="qy", name="qy")
                a = qkp.tile([128, 16], F32, tag="qa", name="qa")
                nc.vector.tensor_scalar(
                    tu[:, :w], src.bitcast(U32), 1, None,
                    op0=ALU.logical_shift_right)
                nc.vector.tensor_sub(y[:, :w].bitcast(U32), Kq[:, :w], tu[:, :w])
                for _ in range(newton):
                    nc.vector.tensor_mul(a[:, :w], y[:, :w], y[:, :w])
                    nc.vector.tensor_mul(a[:, :w], a[:, :w], src)
                    nc.vector.tensor_scalar(
                        a[:, :w], a[:, :w], -0.5, 1.5, op0=ALU.mult, op1=ALU.add)
                    nc.vector.tensor_mul(y[:, :w], y[:, :w], a[:, :w])
                nc.vector.tensor_scalar(dst, y[:, :w], float(scale), None,
                                        op0=ALU.mult)

            # ---------------- z2 streaming machinery
            z2r = z2_d.rearrange("(s n p) d -> s p n d", n=4, p=128)  # 16 stages
            z2st, z2h8 = {}, {}

            def z2_stage(s):
                # cast-DMA: f32 DRAM -> bf16 SBUF (gpsimd software DGE),
                # then one wide bf16 square (DVE 2x mode) + per-block
                # tensor_scalar row-sum accumulation
                st = stg.tile([128, 4, D], BF16, tag="st", name=f"st2_{s}")
                nc.gpsimd.dma_start(out=st[:], in_=z2r[s])
                z2st[s] = st
                sq = sqp.tile([128, 4, D], BF16, tag="sq", name="sq_scr")
                nc.vector.tensor_mul(sq[:], st[:], st[:])
                nc.vector.tensor_reduce(n2sq[:, 4 * s:4 * s + 4], sq[:],
                                        axis=mybir.AxisListType.X, op=ALU.add)

            def z2_norm8(h):
                # blocks 8h..8h+7: rsqrt batch, then wide tensor_tensor with
                # a free-broadcast rn2 operand (AP-scalar tensor_scalar on
                # bf16 is microcode-slow: measured 7us+). DVE writes fp8
                # directly (even stage); Pool writes bf16 (odd stage) which a
                # batched cast-DMA converts off-engine.
                rsqrt_cols(rn2[:, 8 * h:8 * h + 8], n2sq[:, 8 * h:8 * h + 8],
                           8, 16.0, newton=1)
                h8 = h8fp.tile([128, 8, D], FP8, tag="h8f", name=f"h8f{h}")
                hb = hbp.tile([128, 4, D], BF16, tag="hb", name=f"hb{h}")
                se, so = 2 * h, 2 * h + 1
                nc.vector.tensor_tensor(
                    out=h8[:, 0:4, :], in0=z2st[se][:],
                    in1=rn2[:, 4 * se:4 * se + 4].broadcast_to((128, 4, D)),
                    op=ALU.mult)
                nc.gpsimd.tensor_tensor(
                    out=hb[:], in0=z2st[so][:],
                    in1=rn2[:, 4 * so:4 * so + 4].broadcast_to((128, 4, D)),
                    op=ALU.mult)
                nc.gpsimd.dma_start(out=h8[:, 4:8, :], in_=hb[:])
                z2h8[h] = h8

            def z2_T(h):
                # 32 fp8 transposes (stride-2 out) -> 4-bank PSUM tile ->
                # z2T8 cols, moved as a bitcast f32 copy (4 bytes/elem)
                h8 = z2h8.pop(h)
                T = pbig.tile([128, 4, 2048], FP8, tag="big", name=f"T{h}")
                Tv = T[:].rearrange("p k (j t) -> p k j t", t=2)
                if SIM_SAFE:
                    nc.gpsimd.memset(T[:].bitcast(F32), 0.0)
                for k in range(4):
                    for bi in range(8):
                        nc.tensor.transpose(
                            Tv[:, k, bi * 128:(bi + 1) * 128, 0],
                            h8[:, bi, k * 128:(k + 1) * 128], ident8[:])
                nc.vector.tensor_copy(
                    z2T8[:, :, h * 2048:(h + 1) * 2048].bitcast(F32),
                    T[:].bitcast(F32))

            # ---------------- z1 startup (sumsq + fp8 casts on ACT: its
            # table-free Square/Copy run in the pre-Exp idle window)
            z1r = z1_d.rearrange("(b p) d -> p b d", p=128)
            z1h8f = cpool.tile([128, IB, D], FP8, name="z1h8f")

            def z1_prep():
                nc.gpsimd.dma_start(out=st1[:], in_=z1r)
                for b in range(IB):
                    sq = sqp.tile([128, 4, D], BF16, tag="sq", name="sq1_scr")
                    nc.scalar.activation(sq[:, 0, :], st1[:, b, :], AF.Square,
                                         accum_out=n1sq[:, b:b + 1])
                    nc.scalar.copy(z1h8f[:, b, :], st1[:, b, :])
                # 1.25 = 20 (1/T) / 16 (fp8 pre-scale on z2hat)
                rsqrt_cols(rn1[:], n1sq[:], IB, 1.25)
                T = pbig.tile([128, 4, 2048], FP8, tag="big", name="T1")
                Tv = T[:].rearrange("p k (j t) -> p k j t", t=2)
                if SIM_SAFE:
                    nc.gpsimd.memset(T[:].bitcast(F32), 0.0)
                for k in range(4):
                    for bi in range(IB):
                        nc.tensor.transpose(
                            Tv[:, k, bi * 128:(bi + 1) * 128, 0],
                            z1h8f[:, bi, k * 128:(k + 1) * 128], ident8[:])
                nc.vector.tensor_copy(z1T8[:].bitcast(F32), T[:].bitcast(F32))

            # ---------------- main tiles
            def main_tile(ib, jh):
                ps = pbig.tile([128, 2048], F32, tag="big", name=f"mm{ib}_{jh}")
                psv = ps[:].rearrange("p (jq x) -> p jq x", jq=4)
                for kp in range(2):
                    lhsT = z1T8v[:, 2 * kp:2 * kp + 2,
                                 ib * 128:(ib + 1) * 128, 0]
                    for jq in range(4):
                        rhs = z2T8v[:, 2 * kp:2 * kp + 2,
                                    jh * 2048 + jq * 512:jh * 2048 + (jq + 1) * 512,
                                    0]
                        nc.tensor.matmul(
                            psv[:, jq, :], lhsT=lhsT, rhs=rhs,
                            start=(kp == 0), stop=(kp == 1),
                            perf_mode=DR, skip_group_check=True)
                nc.scalar.activation(
                    ps[:], ps[:], AF.Exp, scale=rn1[:, ib:ib + 1],
                    accum_out=esums[:, jh * IB + ib:jh * IB + ib + 1])

            # ---------------- emission
            z2_stage(0)
            z2_stage(1)
            z1_prep()
            z2_norm8(0)
            z2_T(0)
            z2_stage(2)
            z2_stage(3)
            z2_norm8(1)
            z2_T(1)
            for jh in range(NJH):
                for ib in range(IB):
                    main_tile(ib, jh)
                    if jh < NJH - 1:
                        s0 = 4 * (jh + 1)
                        if ib == 0:
                            z2_stage(s0)
                        elif ib == 1:
                            z2_stage(s0 + 1)
                        elif ib == 2:
                            z2_norm8(2 * (jh + 1))
                        elif ib == 3:
                            z2_T(2 * (jh + 1))
                        elif ib == 4:
                            z2_stage(s0 + 2)
                        elif ib == 5:
                            z2_stage(s0 + 3)
                        elif ib == 6:
                            z2_norm8(2 * (jh + 1) + 1)
                        elif ib == 7:
                            z2_T(2 * (jh + 1) + 1)

            # ---------------- logsumexp tail
            nc.vector.reduce_sum(
                stot[:], esums[:].rearrange("p (a b) -> p b a", a=NJH),
                axis=mybir.AxisListType.X)
            nc.scalar.activation(lse_s[:], stot[:], AF.Ln)
            nc.sync.dma_start(out=lse_d[:], in_=lse_s[:])

    nc.compile()
    return nc


_nc = None


def _get_nc():
    global _nc
    if _nc is None:
        _nc = _build()
    return _nc


def kernel(z1: np.ndarray, z2: np.ndarray, _trace: bool = False, **_):
    nc = _get_nc()
    z1 = np.ascontiguousarray(z1, dtype=np.float32)
    z2 = np.ascontiguousarray(z2, dtype=np.float32)
    in_maps = [
        {"z1s": z1[c * NS:(c + 1) * NS], "z2": z2} for c in range(C)
    ]
    res = run_bass_kernel_spmd(nc, in_maps, list(range(C)), trace=_trace)
    total = 0.0
    for c in range(C):
        total += res.results[c]["lse"].astype(np.float64).sum()
    out = np.float32(-(total / N))
    if _trace:
        return out, res
    return out


# revision 9
# speedup vs baseline: 2.5057x; 1.1163x over previous
"""Contrastive loss (InfoNCE-style logsumexp of cosine-similarity matrix) on
8 Trainium2 NeuronCores — fp8 DoubleRow edition.

loss = -mean_i logsumexp_j( cos(z1_i, z2_j) / 0.05 ),  z1,z2: [8192, 512] f32

Strategy: shard z1 row-wise (1024 rows/core), replicate z2. All matmul
operands in float8e4 (e4m3): numerics validated at ~5e-5 rel err (tolerance
2e-2) because per-element quantization noise averages out over the K=512 dot
and the 8192-term logsumexp.

Per core:
  1. z2 streams in via gpsimd cast-DMAs (f32 DRAM -> bf16 SBUF, RNE);
     per-stage bf16 squares + per-block tensor_scalar-with-accum row sums
     on DVE (2x 16-bit mode); 1/||z2|| via quake-rsqrt (bitcast + Newton
     on DVE — keeps ACT's activation table untouched); normalize in bf16
     scaled by 16 (so values sit in e4m3's normal range), then ONE batched
     SBUF->SBUF cast-DMA per 8 blocks converts bf16 -> fp8 off-engine
     (vector-engine fp8 converts are microcode-slow: measured 4-8us/block),
  2. fp8 PE transposes (1 cyc/row): hw writes fp8 transpose outputs at
     element step 2, so transposed operands use a stride-2 interleaved
     layout end-to-end; PSUM -> SBUF drains are bitcast-f32 copies
     (4 bytes/cycle/lane),
  3. sim block-rows: DoubleRow fp8 matmuls (two k-tiles per instruction,
     2x bf16 throughput) into [128, 2048] 4-bank PSUM tiles, two tiles
     ping-pong across the 8 banks; transposes for the next j-supergroup
     slot into the same pool ring between main tiles,
  4. one ACT Exp per [128, 2048] tile, in place, scale = 1.25/||z1_i||
     (= 20/(16*||z1_i||), folding the fp8 pre-scale), fused row-sum via
     accum_out. Exp/Ln/Square/Copy share one act table -> one table load;
     z1's prep (sum-of-squares + fp8 casts) runs on ACT in the startup
     window before the first Exp.
  5. reduce + Ln -> per-row lse [128, 8] -> DRAM. Host gathers, -mean.
"""
import sys

sys.path.insert(0, "/opt/trn_rl_repo")
import numpy as np
import concourse.bacc as bacc
import concourse.mybir as mybir
from concourse import tile, masks
from concourse.bass_utils import run_bass_kernel_spmd

F32 = mybir.dt.float32
BF16 = mybir.dt.bfloat16
FP8 = mybir.dt.float8e4
U32 = mybir.dt.uint32
AF = mybir.ActivationFunctionType
ALU = mybir.AluOpType
DR = mybir.MatmulPerfMode.DoubleRow

N, D, C = 8192, 512, 8
NS = N // C            # 1024 z1 rows per core
IB = NS // 128         # 8 i-blocks per core
NJH = 4                # j-supergroups of 2048 columns
QK = 0x5F3759DF        # quake rsqrt magic

# CoreSim-only: zero-fill transpose PSUM tiles so the bitcast copy over the
# stride-2 gaps doesn't trip the interpreter's uninitialized-read check.
# Hardware never reads those bytes meaningfully; keep False for real runs.
SIM_SAFE = False


def _build():
    nc = bacc.Bacc("TRN2", target_bir_lowering=False, debug=False, num_devices=C)
    z1_d = nc.dram_tensor("z1s", [NS, D], F32, kind="ExternalInput").ap()
    z2_d = nc.dram_tensor("z2", [N, D], F32, kind="ExternalInput").ap()
    lse_d = nc.dram_tensor("lse", [128, IB], F32, kind="ExternalOutput").ap()

    with tile.TileContext(nc) as tc:
        with (
            tc.tile_pool(name="const", bufs=1) as cpool,
            tc.tile_pool(name="stage", bufs=3) as stg,
            tc.tile_pool(name="hb", bufs=2) as hbp,
            tc.tile_pool(name="h8f", bufs=2) as h8fp,
            tc.tile_pool(name="sq", bufs=2) as sqp,
            tc.tile_pool(name="qk", bufs=3) as qkp,
            tc.tile_pool(name="pbig", bufs=2, space="PSUM") as pbig,
        ):
            ident8 = cpool.tile([128, 128], FP8)
            masks.make_identity(nc, ident8[:])
            Kq = cpool.tile([128, 16], U32, name="Kq")
            nc.gpsimd.memset(Kq[:], QK)

            # stride-2 interleaved transposed operands (see module docstring)
            z2T8 = cpool.tile([128, 4, 2 * N], FP8, name="z2T8")
            z1T8 = cpool.tile([128, 4, 2 * NS], FP8, name="z1T8")
            z2T8v = z2T8[:].rearrange("p k (j t) -> p k j t", t=2)
            z1T8v = z1T8[:].rearrange("p k (j t) -> p k j t", t=2)

            n2sq = cpool.tile([128, 64], F32, name="n2sq")
            rn2 = cpool.tile([128, 64], F32, name="rn2")
            n1sq = cpool.tile([128, IB], F32, name="n1sq")
            rn1 = cpool.tile([128, IB], F32, name="rn1")
            esums = cpool.tile([128, NJH * IB], F32, name="esums")
            stot = cpool.tile([128, IB], F32, name="stot")
            lse_s = cpool.tile([128, IB], F32, name="lse_s")
            st1 = cpool.tile([128, IB, D], BF16, name="st1")  # z1 bf16 stage

            def rsqrt_cols(dst, src, w, scale, newton=2):
                # dst = scale / sqrt(src) entirely on DVE (no ACT table load):
                # quake initial guess + Newton steps (1 step ~1.7e-3 rel err,
                # 2 steps ~5e-6; either is far inside the fp8 noise floor)
                tu = qkp.tile([128, 16], U32, tag="qu", name="tu")
                y = qkp.tile([128, 16], F32, tag="qy", name="qy")
                a = qkp.tile([128, 16], F32, tag="qa", name="qa")
                nc.vector.tensor_scalar(
                    tu[:, :w], src.bitcast(U32), 1, None,
                    op0=ALU.logical_shift_right)
                nc.vector.tensor_sub(y[:, :w].bitcast(U32), Kq[:, :w], tu[:, :w])
                for _ in range(newton):
                    nc.vector.tensor_mul(a[:, :w], y[:, :w], y[:, :w])
                    nc.vector.tensor_mul(a[:, :w], a[:, :w], src)
                    nc.vector.tensor_scalar(
                        a[:, :w], a[:, :w], -0.5, 1.5, op0=ALU.mult, op1=ALU.add)
                    nc.vector.tensor_mul(y[:, :w], y[:, :w], a[:, :w])
                nc.vector.tensor_scalar(dst, y[:, :w], float(scale), None,
                                        op0=ALU.mult)

            # ---------------- z2 streaming machinery
            z2r = z2_d.rearrange("(s n p) d -> s p n d", n=4, p=128)  # 16 stages
            z2st, z2h8 = {}, {}

            def z2_stage(s):
                # cast-DMA: f32 DRAM -> bf16 SBUF (gpsimd software DGE),
                # then one wide bf16 square (DVE 2x mode) + per-block
                # tensor_scalar row-sum accumulation
                st = stg.tile([128, 4, D], BF16, tag="st", name=f"st2_{s}")
                nc.gpsimd.dma_start(out=st[:], in_=z2r[s])
                z2st[s] = st
                sq = sqp.tile([128, 4, D], BF16, tag="sq", name="sq_scr")
                nc.vector.tensor_mul(sq[:], st[:], st[:])
                nc.vector.tensor_reduce(n2sq[:, 4 * s:4 * s + 4], sq[:],
                                        axis=mybir.AxisListType.X, op=ALU.add)

            def z2_norm8(h):
                # blocks 8h..8h+7: rsqrt batch, then wide tensor_tensor with
                # a free-broadcast rn2 operand (AP-scalar tensor_scalar on
                # bf16 is microcode-slow: measured 7us+). DVE writes fp8
                # directly (even stage); Pool writes bf16 (odd stage) which a
                # batched cast-DMA converts off-engine.
                rsqrt_cols(rn2[:, 8 * h:8 * h + 8], n2sq[:, 8 * h:8 * h + 8],
                           8, 16.0, newton=1)
                h8 = h8fp.tile([128, 8, D], FP8, tag="h8f", name=f"h8f{h}")
                for g in range(2):
                    s = 2 * h + g
                    nc.vector.tensor_tensor(
                        out=h8[:, 4 * g:4 * g + 4, :], in0=z2st[s][:],
                        in1=rn2[:, 4 * s:4 * s + 4].broadcast_to((128, 4, D)),
                        op=ALU.mult)
                z2h8[h] = h8

            def z2_T(h):
                # 32 fp8 transposes (stride-2 out) -> 4-bank PSUM tile ->
                # z2T8 cols, moved as a bitcast f32 copy (4 bytes/elem)
                h8 = z2h8.pop(h)
                T = pbig.tile([128, 4, 2048], FP8, tag="big", name=f"T{h}")
                Tv = T[:].rearrange("p k (j t) -> p k j t", t=2)
                if SIM_SAFE:
                    nc.gpsimd.memset(T[:].bitcast(F32), 0.0)
                for k in range(4):
                    for bi in range(8):
                        nc.tensor.transpose(
                            Tv[:, k, bi * 128:(bi + 1) * 128, 0],
                            h8[:, bi, k * 128:(k + 1) * 128], ident8[:])
                nc.vector.tensor_copy(
                    z2T8[:, :, h * 2048:(h + 1) * 2048].bitcast(F32),
                    T[:].bitcast(F32))

            # ---------------- z1 startup (sumsq + fp8 casts on ACT: its
            # table-free Square/Copy run in the pre-Exp idle window)
            z1r = z1_d.rearrange("(b p) d -> p b d", p=128)
            z1h8f = cpool.tile([128, IB, D], FP8, name="z1h8f")

            def z1_prep():
                nc.gpsimd.dma_start(out=st1[:], in_=z1r)
                for b in range(IB):
                    sq = sqp.tile([128, 4, D], BF16, tag="sq", name="sq1_scr")
                    nc.scalar.activation(sq[:, 0, :], st1[:, b, :], AF.Square,
                                         accum_out=n1sq[:, b:b + 1])
                    nc.scalar.copy(z1h8f[:, b, :], st1[:, b, :])
                # 1.25 = 20 (1/T) / 16 (fp8 pre-scale on z2hat)
                rsqrt_cols(rn1[:], n1sq[:], IB, 1.25)
                T = pbig.tile([128, 4, 2048], FP8, tag="big", name="T1")
                Tv = T[:].rearrange("p k (j t) -> p k j t", t=2)
                if SIM_SAFE:
                    nc.gpsimd.memset(T[:].bitcast(F32), 0.0)
                for k in range(4):
                    for bi in range(IB):
                        nc.tensor.transpose(
                            Tv[:, k, bi * 128:(bi + 1) * 128, 0],
                            z1h8f[:, bi, k * 128:(k + 1) * 128], ident8[:])
                nc.vector.tensor_copy(z1T8[:].bitcast(F32), T[:].bitcast(F32))

            # ---------------- main tiles
            def main_tile(ib, jh):
                ps = pbig.tile([128, 2048], F32, tag="big", name=f"mm{ib}_{jh}")
                psv = ps[:].rearrange("p (jq x) -> p jq x", jq=4)
                for kp in range(2):
                    lhsT = z1T8v[:, 2 * kp:2 * kp + 2,
                                 ib * 128:(ib + 1) * 128, 0]
                    for jq in range(4):
                        rhs = z2T8v[:, 2 * kp:2 * kp + 2,
                                    jh * 2048 + jq * 512:jh * 2048 + (jq + 1) * 512,
                                    0]
                        nc.tensor.matmul(
                            psv[:, jq, :], lhsT=lhsT, rhs=rhs,
                            start=(kp == 0), stop=(kp == 1),
                            perf_mode=DR, skip_group_check=True)
                nc.scalar.activation(
                    ps[:], ps[:], AF.Exp, scale=rn1[:, ib:ib + 1],
                    accum_out=esums[:, jh * IB + ib:jh * IB + ib + 1])

            # ---------------- emission
            z2_stage(0)
            z2_stage(1)
            z1_prep()
            z2_norm8(0)
            z2_T(0)
            z2_stage(2)
            z2_stage(3)
            z2_norm8(1)
            z2_T(1)
            for jh in range(NJH):
                for ib in range(IB):
                    main_tile(ib, jh)
                    if jh < NJH - 1:
                        s0 = 4 * (jh + 1)
                        if ib == 0:
                            z2_stage(s0)
                            z2_stage(s0 + 1)
                        elif ib == 1:
                            z2_norm8(2 * (jh + 1))
                        elif ib == 2:
                            z2_T(2 * (jh + 1))
                        elif ib == 3:
                            z2_stage(s0 + 2)
                        elif ib == 4:
                            z2_stage(s0 + 3)
                        elif ib == 5:
                            z2_norm8(2 * (jh + 1) + 1)
                        elif ib == 6:
                            z2_T(2 * (jh + 1) + 1)

            # ---------------- logsumexp tail
            nc.vector.reduce_sum(
                stot[:], esums[:].rearrange("p (a b) -> p b a", a=NJH),
                axis=mybir.AxisListType.X)
            nc.scalar.activation(lse_s[:], stot[:], AF.Ln)
            nc.sync.dma_start(out=lse_d[:], in_=lse_s[:])

    nc.compile()
    return nc


_nc = None


def _get_nc():
    global _nc
    if _nc is None:
        _nc = _build()
    return _nc


def kernel(z1: np.ndarray, z2: np.ndarray, _trace: bool = False, **_):
    nc = _get_nc()
    z1 = np.ascontiguousarray(z1, dtype=np.float32)
    z2 = np.ascontiguousarray(z2, dtype=np.float32)
    in_maps = [
        {"z1s": z1[c * NS:(c + 1) * NS], "z2": z2} for c in range(C)
    ]
    res = run_bass_kernel_spmd(nc, in_maps, list(range(C)), trace=_trace)
    total = 0.0
    for c in range(C):
        total += res.results[c]["lse"].astype(np.float64).sum()
    out = np.float32(-(total / N))
    if _trace:
        return out, res
    return out
